# revision 3
# baseline (speedup 1.0000x reference)
"""Trainium2 Bass kernel for nn_AttentionLayer_77309411672.

Math (per (b, h) head, 8 heads = 8 cores, no collectives):
  x        : [64, 4096]  slice queries[b, :, :, h]
  weight-normed 1x1 projections fused on host:
    G_aug  [65, 65] : S~^T = (G^T x_aug)^T x_aug  gives scaled scores^T
                      (folds Wq^T Wk, the 1/sqrt(D) scale, and q/k biases)
    WV_aug [65, 64] : vt = x_aug^T WV_aug gives (Wo Wv x + Wo bv)^T
                      (folds Wo into the V projection; valid since softmax
                       rows sum to 1)
  A^T = exp(S~^T)  (no max subtraction needed: |S~| <~ 8 for these inputs)
  o2 = [vt | 1]^T A^T  -> rows 0:64 unnormalized output, row 64 = softmax
       denominators (ones-column trick)
  out = x + o2[:64] * (1/o2[64]) + bo

Device layout: scores computed transposed ([s, l]) so softmax runs along
the free axis; V^T tiles act as matmul stationary so PV needs no
transposes; denominators come free as an extra stationary column.
"""

import numpy as np

D = 64
L = 4096
B = 2
V = 4
NCORES = 8
LSEC = 1024          # l columns per section (psum: [128, LSEC] f32 = 2 banks)
NSEC = L // LSEC
SCH = 128            # s-chunk (partition tile)
NSC = L // SCH

_COMPILED = None


def _build_nc():
    import concourse.bacc as bacc
    import concourse.mybir as mybir
    from concourse import tile

    f32 = mybir.dt.float32
    bf16 = mybir.dt.bfloat16
    Exp = mybir.ActivationFunctionType.Exp
    add = mybir.AluOpType.add
    mult = mybir.AluOpType.mult

    nc = bacc.Bacc(
        "TRN2",
        target_bir_lowering=False,
        debug=False,
        enable_asserts=True,
        num_devices=NCORES,
    )
    x_d = nc.declare_dram_parameter("x", [D, L], f32, isOutput=False)
    xb_d = nc.declare_dram_parameter("xb", [D + 1, L], bf16, isOutput=False)
    g_d = nc.declare_dram_parameter("gaug", [D + 1, D + 1], bf16, isOutput=False)
    wv_d = nc.declare_dram_parameter("wvaug", [D + 1, D], bf16, isOutput=False)
    br_d = nc.declare_dram_parameter("bres", [D, 1], f32, isOutput=False)
    out_d = nc.declare_dram_parameter("out", [D, L], f32, isOutput=True)

    with tile.TileContext(nc) as tc:
        with (
            tc.tile_pool(name="const", bufs=1) as cpool,
            tc.tile_pool(name="big", bufs=1) as bpool,
        ):
            x_f = bpool.tile([D, L], f32)
            xb = bpool.tile([D + 1, L], bf16)
            kp = bpool.tile([D + 1, L], bf16)
            vt = bpool.tile([128, NSC * (D + 1)], bf16)
            recip = bpool.tile([1, L], f32)
            g_t = cpool.tile([D + 1, D + 1], bf16)
            wv_t = cpool.tile([D + 1, D], bf16)
            br_t = cpool.tile([D, 1], f32)
            ones = cpool.tile([1, D], f32)

            # ---- loads ----
            for c in range(4):
                s = slice(c * (L // 4), (c + 1) * (L // 4))
                nc.sync.dma_start(xb[:, s], xb_d[:, s])
            for c in range(4):
                s = slice(c * (L // 4), (c + 1) * (L // 4))
                nc.sync.dma_start(x_f[:, s], x_d[:, s])
            nc.sync.dma_start(g_t[:], g_d[:, :])
            nc.sync.dma_start(wv_t[:], wv_d[:, :])
            nc.sync.dma_start(br_t[:], br_d[:, :])
            nc.vector.memset(ones[:], 1.0)
            nc.vector.memset(vt[:], 1.0)

            # ---- projections ----
            with tc.tile_pool(name="hps", bufs=4, space="PSUM") as hps:
                # k' projection: kp[m, s] = sum_i G[i, m] x_aug[i, s]
                for c in range(8):
                    s = slice(c * 512, (c + 1) * 512)
                    ps = hps.tile([D + 1, 512], f32, tag="h")
                    nc.tensor.matmul(ps[:], g_t[:], xb[:, s], start=True, stop=True)
                    nc.scalar.copy(kp[:, s], ps[:])
                # vt' projection: vt[s, e] = sum_i x_aug[i, s] WV[i, e]
                for grp in range(4):
                    ps = hps.tile([128, 512], f32, tag="h")
                    for j8 in range(8):
                        j = grp * 8 + j8
                        nc.tensor.matmul(
                            ps[:, j8 * 64 : j8 * 64 + 64],
                            xb[:, j * SCH : (j + 1) * SCH],
                            wv_t[:],
                            start=True,
                            stop=True,
                        )
                    dst = (
                        vt[:, grp * 520 : (grp + 1) * 520]
                        .rearrange("p (j c) -> p j c", c=D + 1)[:, :, 0:D]
                    )
                    src = ps[:].rearrange("p (j c) -> p j c", c=D)
                    nc.vector.tensor_copy(out=dst, in_=src)

            # ---- attention pipeline + fused epilogue ----
            with (
                tc.tile_pool(name="stp", bufs=2, space="PSUM") as stp,
                tc.tile_pool(name="o2p", bufs=2, space="PSUM") as o2p,
                tc.tile_pool(name="atp", bufs=3) as atp,
                tc.tile_pool(name="tsb", bufs=4) as tsb,
            ):
                for sec in range(NSEC):
                    lw = sec * LSEC
                    o2 = o2p.tile([D + 1, LSEC], f32)
                    for j in range(NSC):
                        st = stp.tile([128, LSEC], f32, tag="st")
                        for h in range(LSEC // 512):
                            hs = slice(h * 512, (h + 1) * 512)
                            nc.tensor.matmul(
                                st[:, hs],
                                kp[:, j * SCH : (j + 1) * SCH],
                                xb[:, lw + h * 512 : lw + (h + 1) * 512],
                                start=True,
                                stop=True,
                            )
                        at = atp.tile([128, LSEC], bf16)
                        nc.scalar.activation(at[:], st[:], Exp)
                        for h in range(LSEC // 512):
                            hs = slice(h * 512, (h + 1) * 512)
                            nc.tensor.matmul(
                                o2[:, hs],
                                vt[:, j * 65 : (j + 1) * 65],
                                at[:, hs],
                                start=(j == 0),
                                stop=(j == NSC - 1),
                                skip_group_check=True,
                            )
                    # epilogue for this section (overlaps next section)
                    nc.vector.reciprocal(recip[:, lw : lw + LSEC], o2[D : D + 1, :])
                    for c in range(LSEC // 512):
                        lo = lw + c * 512
                        co = slice(c * 512, (c + 1) * 512)
                        pr = stp.tile([128, 512], f32, tag="st")
                        nc.tensor.matmul(
                            pr[0:D, :],
                            ones[:],
                            recip[:, lo : lo + 512],
                            start=True,
                            stop=True,
                        )
                        rb = tsb.tile([D, 512], f32, tag="rb")
                        nc.scalar.copy(rb[:], pr[0:D, :])
                        y1 = tsb.tile([D, 512], f32, tag="y1")
                        nc.vector.tensor_tensor(
                            out=y1[:], in0=o2[0:D, co], in1=rb[:], op=mult
                        )
                        res = tsb.tile([D, 512], f32, tag="res")
                        nc.vector.scalar_tensor_tensor(
                            out=res[:],
                            in0=y1[:],
                            scalar=br_t[:, 0:1],
                            in1=x_f[:, lo : lo + 512],
                            op0=add,
                            op1=add,
                        )
                        nc.sync.dma_start(out_d[:, lo : lo + 512], res[:])
    nc.compile()
    return nc


def _get_compiled():
    global _COMPILED
    if _COMPILED is None:
        _COMPILED = _build_nc()
    return _COMPILED


def _host_prep(q_v, q_g, q_b, k_v, k_g, k_b, v_v, v_g, v_b, o_v, o_g, o_b):
    import ml_dtypes

    scale = np.float32(1.0 / np.sqrt(D))

    def wn(v, g):
        v = np.asarray(v, np.float64)
        g = np.asarray(g, np.float64)
        nrm = np.sqrt((v * v).sum(1, keepdims=True))
        return (g[:, None] / nrm) * v

    wq, wk, wv, wo = wn(q_v, q_g), wn(k_v, k_g), wn(v_v, v_g), wn(o_v, o_g)
    bq = np.asarray(q_b, np.float64)
    bk = np.asarray(k_b, np.float64)
    bv = np.asarray(v_b, np.float64)
    bo = np.asarray(o_b, np.float64)

    G = np.zeros((D + 1, D + 1), np.float64)
    G[:D, :D] = (scale * wq.T @ wk).T
    G[D, :D] = scale * wq.T @ bk
    G[:D, D] = scale * wk.T @ bq
    G[D, D] = scale * (bq @ bk)

    WV = np.zeros((D + 1, D), np.float64)
    WV[:D, :] = (wo @ wv).T
    WV[D, :] = wo @ bv

    gaug = G.astype(ml_dtypes.bfloat16)
    wvaug = WV.astype(ml_dtypes.bfloat16)
    bres = np.ascontiguousarray(bo.astype(np.float32).reshape(D, 1))
    return gaug, wvaug, bres


def kernel(queries, q_v, q_g, q_b, k_v, k_g, k_b, v_v, v_g, v_b, o_v, o_g, o_b):
    import ml_dtypes
    from concourse.bass_utils import run_bass_kernel_spmd

    queries = np.asarray(queries, np.float32)
    gaug, wvaug, bres = _host_prep(
        q_v, q_g, q_b, k_v, k_g, k_b, v_v, v_g, v_b, o_v, o_g, o_b
    )

    in_maps = []
    for i in range(NCORES):
        b, h = divmod(i, V)
        x = np.ascontiguousarray(queries[b, :, :, h])  # [64, 4096] f32
        xb = np.empty((D + 1, L), ml_dtypes.bfloat16)
        xb[:D, :] = x.astype(ml_dtypes.bfloat16)
        xb[D, :] = np.ones((L,), ml_dtypes.bfloat16)
        in_maps.append(
            {"x": x, "xb": xb, "gaug": gaug, "wvaug": wvaug, "bres": bres}
        )

    nc = _get_compiled()
    res = run_bass_kernel_spmd(nc, in_maps, core_ids=list(range(NCORES)))

    out = np.empty((B, D, L, V), np.float32)
    for i in range(NCORES):
        b, h = divmod(i, V)
        out[b, :, :, h] = res.results[i]["out"]
    return out


# revision 5
# speedup vs baseline: 1.2174x; 1.2174x over previous
"""Trainium2 Bass kernel for nn_AttentionLayer_77309411672.

Math (per (b, h) head, 8 heads = 8 cores, no collectives):
  x        : [64, 4096]  slice queries[b, :, :, h]
  weight-normed 1x1 projections fused on host:
    G_aug  [65, 65] : S~^T = (G^T x_aug)^T x_aug  gives scaled scores^T
                      (folds Wq^T Wk, the 1/sqrt(D) scale, and q/k biases)
    WV_aug [65, 64] : vt = x_aug^T WV_aug gives (Wo Wv x + Wo bv)^T
                      (folds Wo into the V projection; valid since softmax
                       rows sum to 1)
  A^T = exp(S~^T)  (no max subtraction needed: |S~| <~ 8 for these inputs)
  o2 = [vt | 1]^T A^T  -> rows 0:64 unnormalized output, row 64 = softmax
       denominators (ones-column trick)
  out = x + o2[:64] * (1/o2[64]) + bo

Device layout: scores computed transposed ([s, l]) so softmax runs along
the free axis; V^T tiles act as matmul stationary so PV needs no
transposes; denominators come free as an extra stationary column. The
per-section epilogue (reciprocal, partition-broadcast, normalize,
residual) runs entirely on DVE + GpSimd + DMA so it never blocks the
TensorE/ScalarE pipeline of the next section.
"""

import numpy as np

D = 64
L = 4096
B = 2
V = 4
NCORES = 8
LSEC = 1024          # l columns per section (psum: [128, LSEC] f32 = 2 banks)
NSEC = L // LSEC
SCH = 128            # s-chunk (partition tile)
NSC = L // SCH

_COMPILED = None


def _build_nc():
    import concourse.bacc as bacc
    import concourse.mybir as mybir
    from concourse import tile

    f32 = mybir.dt.float32
    bf16 = mybir.dt.bfloat16
    Exp = mybir.ActivationFunctionType.Exp
    add = mybir.AluOpType.add
    mult = mybir.AluOpType.mult

    nc = bacc.Bacc(
        "TRN2",
        target_bir_lowering=False,
        debug=False,
        enable_asserts=True,
        num_devices=NCORES,
    )
    x_d = nc.declare_dram_parameter("x", [D, L], f32, isOutput=False)
    xb_d = nc.declare_dram_parameter("xb", [D + 1, L], bf16, isOutput=False)
    g_d = nc.declare_dram_parameter("gaug", [D + 1, D + 1], bf16, isOutput=False)
    wv_d = nc.declare_dram_parameter("wvaug", [D + 1, D], bf16, isOutput=False)
    br_d = nc.declare_dram_parameter("bres", [D, 1], f32, isOutput=False)
    out_d = nc.declare_dram_parameter("out", [D, L], f32, isOutput=True)

    with tile.TileContext(nc) as tc:
        with (
            tc.tile_pool(name="const", bufs=1) as cpool,
            tc.tile_pool(name="big", bufs=1) as bpool,
        ):
            x_f = bpool.tile([D, L], f32)
            xbq = [
                bpool.tile(
                    [D + 1, LSEC], bf16, name=f"xbq{q}", tag=f"xbq{q}"
                )
                for q in range(NSEC)
            ]
            kp = bpool.tile([D + 1, L], bf16)
            vt = bpool.tile([128, NSC * (D + 1)], bf16)
            g_t = cpool.tile([D + 1, D + 1], bf16)
            wv_t = cpool.tile([D + 1, D], bf16)
            br_t = cpool.tile([D, 1], f32)
            warm = cpool.tile([1, 64], f32)
            warm_o = cpool.tile([1, 64], f32)

            # warm the ACT exp table while DMAs run
            nc.vector.memset(warm[:], 0.0)
            nc.scalar.activation(warm_o[:], warm[:], Exp)

            # ---- loads ----
            for q in range(NSEC):
                nc.sync.dma_start(
                    xbq[q][:], xb_d[:, q * LSEC : (q + 1) * LSEC]
                )
            for c in range(4):
                s = slice(c * (L // 4), (c + 1) * (L // 4))
                nc.sync.dma_start(x_f[:, s], x_d[:, s])
            nc.sync.dma_start(g_t[:], g_d[:, :])
            nc.sync.dma_start(wv_t[:], wv_d[:, :])
            nc.sync.dma_start(br_t[:], br_d[:, :])
            nc.vector.memset(vt[:], 1.0)

            # ---- projections ----
            with tc.tile_pool(name="hps", bufs=4, space="PSUM") as hps:
                # k' projection: kp[m, s] = sum_i G[i, m] x_aug[i, s]
                for c in range(8):
                    q, hh = divmod(c, 2)
                    ps = hps.tile([D + 1, 512], f32, tag="h")
                    nc.tensor.matmul(
                        ps[:],
                        g_t[:],
                        xbq[q][:, hh * 512 : (hh + 1) * 512],
                        start=True,
                        stop=True,
                    )
                    eng = nc.scalar if c % 2 == 0 else nc.vector
                    if c % 2 == 0:
                        nc.scalar.copy(kp[:, c * 512 : (c + 1) * 512], ps[:])
                    else:
                        nc.vector.tensor_copy(
                            out=kp[:, c * 512 : (c + 1) * 512], in_=ps[:]
                        )
                # vt' projection: vt[s, e] = sum_i x_aug[i, s] WV[i, e]
                for grp in range(4):
                    ps = hps.tile([128, 512], f32, tag="h")
                    for j8 in range(8):
                        j = grp * 8 + j8
                        q, r = divmod(j * SCH, LSEC)
                        nc.tensor.matmul(
                            ps[:, j8 * 64 : j8 * 64 + 64],
                            xbq[q][:, r : r + SCH],
                            wv_t[:],
                            start=True,
                            stop=True,
                        )
                    dst = (
                        vt[:, grp * 520 : (grp + 1) * 520]
                        .rearrange("p (j c) -> p j c", c=D + 1)[:, :, 0:D]
                    )
                    src = ps[:].rearrange("p (j c) -> p j c", c=D)
                    nc.vector.tensor_copy(out=dst, in_=src)

            # ---- attention pipeline + fused epilogue ----
            with (
                tc.tile_pool(name="stp", bufs=2, space="PSUM") as stp,
                tc.tile_pool(name="o2p", bufs=2, space="PSUM") as o2p,
                tc.tile_pool(name="atp", bufs=3) as atp,
                tc.tile_pool(name="tsb", bufs=4) as tsb,
            ):
                for sec in range(NSEC):
                    lw = sec * LSEC
                    o2 = o2p.tile([D + 1, LSEC], f32)
                    for j in range(NSC):
                        st = stp.tile([128, LSEC], f32, tag="st")
                        for h in range(LSEC // 512):
                            hs = slice(h * 512, (h + 1) * 512)
                            nc.tensor.matmul(
                                st[:, hs],
                                kp[:, j * SCH : (j + 1) * SCH],
                                xbq[sec][:, hs],
                                start=True,
                                stop=True,
                            )
                        at = atp.tile([128, LSEC], bf16)
                        nc.scalar.activation(at[:], st[:], Exp)
                        for h in range(LSEC // 512):
                            hs = slice(h * 512, (h + 1) * 512)
                            nc.tensor.matmul(
                                o2[:, hs],
                                vt[:, j * 65 : (j + 1) * 65],
                                at[:, hs],
                                start=(j == 0),
                                stop=(j == NSC - 1),
                                skip_group_check=True,
                            )
                    # epilogue for this section: DVE + GpSimd + DMA only,
                    # overlaps the next section's TensorE/ScalarE pipeline.
                    for c in range(LSEC // 512):
                        lo = lw + c * 512
                        co = slice(c * 512, (c + 1) * 512)
                        rc = tsb.tile([1, 512], f32, tag="rc")
                        nc.vector.reciprocal(rc[:], o2[D : D + 1, co])
                        rb = tsb.tile([D, 512], f32, tag="rb")
                        nc.gpsimd.partition_broadcast(rb[:], rc[:])
                        y1 = tsb.tile([D, 512], f32, tag="y1")
                        nc.vector.tensor_tensor(
                            out=y1[:], in0=o2[0:D, co], in1=rb[:], op=mult
                        )
                        res = tsb.tile([D, 512], f32, tag="res")
                        nc.vector.scalar_tensor_tensor(
                            out=res[:],
                            in0=y1[:],
                            scalar=br_t[:, 0:1],
                            in1=x_f[:, lo : lo + 512],
                            op0=add,
                            op1=add,
                        )
                        nc.sync.dma_start(out_d[:, lo : lo + 512], res[:])
    nc.compile()
    return nc


def _get_compiled():
    global _COMPILED
    if _COMPILED is None:
        _COMPILED = _build_nc()
    return _COMPILED


def _host_prep(q_v, q_g, q_b, k_v, k_g, k_b, v_v, v_g, v_b, o_v, o_g, o_b):
    import ml_dtypes

    scale = np.float32(1.0 / np.sqrt(D))

    def wn(v, g):
        v = np.asarray(v, np.float64)
        g = np.asarray(g, np.float64)
        nrm = np.sqrt((v * v).sum(1, keepdims=True))
        return (g[:, None] / nrm) * v

    wq, wk, wv, wo = wn(q_v, q_g), wn(k_v, k_g), wn(v_v, v_g), wn(o_v, o_g)
    bq = np.asarray(q_b, np.float64)
    bk = np.asarray(k_b, np.float64)
    bv = np.asarray(v_b, np.float64)
    bo = np.asarray(o_b, np.float64)

    G = np.zeros((D + 1, D + 1), np.float64)
    G[:D, :D] = (scale * wq.T @ wk).T
    G[D, :D] = scale * wq.T @ bk
    G[:D, D] = scale * wk.T @ bq
    G[D, D] = scale * (bq @ bk)

    WV = np.zeros((D + 1, D), np.float64)
    WV[:D, :] = (wo @ wv).T
    WV[D, :] = wo @ bv

    gaug = G.astype(ml_dtypes.bfloat16)
    wvaug = WV.astype(ml_dtypes.bfloat16)
    bres = np.ascontiguousarray(bo.astype(np.float32).reshape(D, 1))
    return gaug, wvaug, bres


def _make_in_maps(queries, gaug, wvaug, bres):
    import ml_dtypes

    in_maps = []
    for i in range(NCORES):
        b, h = divmod(i, V)
        x = np.ascontiguousarray(queries[b, :, :, h])  # [64, 4096] f32
        xb = np.empty((D + 1, L), ml_dtypes.bfloat16)
        xb[:D, :] = x.astype(ml_dtypes.bfloat16)
        xb[D, :] = np.ones((L,), ml_dtypes.bfloat16)
        in_maps.append(
            {"x": x, "xb": xb, "gaug": gaug, "wvaug": wvaug, "bres": bres}
        )
    return in_maps


def kernel(queries, q_v, q_g, q_b, k_v, k_g, k_b, v_v, v_g, v_b, o_v, o_g, o_b):
    from concourse.bass_utils import run_bass_kernel_spmd

    queries = np.asarray(queries, np.float32)
    gaug, wvaug, bres = _host_prep(
        q_v, q_g, q_b, k_v, k_g, k_b, v_v, v_g, v_b, o_v, o_g, o_b
    )
    in_maps = _make_in_maps(queries, gaug, wvaug, bres)

    nc = _get_compiled()
    res = run_bass_kernel_spmd(nc, in_maps, core_ids=list(range(NCORES)))

    out = np.empty((B, D, L, V), np.float32)
    for i in range(NCORES):
        b, h = divmod(i, V)
        out[b, :, :, h] = res.results[i]["out"]
    return out


# revision 11
# speedup vs baseline: 1.2578x; 1.0332x over previous
"""Trainium2 Bass kernel for nn_AttentionLayer_77309411672.

Math (per (b, h) head, 8 heads = 8 cores, no collectives):
  x        : [64, 4096]  slice queries[b, :, :, h]
  weight-normed 1x1 projections fused on host:
    G_aug  [65, 65] : S~^T = (G^T x_aug)^T x_aug  gives scaled scores^T
                      (folds Wq^T Wk, the 1/sqrt(D) scale, and q/k biases)
    WV_aug [65, 64] : vt = x_aug^T WV_aug gives (Wo Wv x + Wo bv)^T
                      (folds Wo into the V projection; valid since softmax
                       rows sum to 1)
  A^T = exp(S~^T)  (no max subtraction needed: |S~| <~ 8 for these inputs)
  o2 = [vt | 1]^T A^T  -> rows 0:64 unnormalized output, row 64 = softmax
       denominators (ones-column trick)
  out = x + o2[:64] * (1/o2[64]) + bo

Device layout: scores computed transposed ([s, l]) so softmax runs along
the free axis; V^T tiles act as matmul stationary so PV needs no
transposes; denominators come free as an extra stationary column. The
per-section epilogue (reciprocal, partition-broadcast, normalize,
residual) runs entirely on DVE + GpSimd + DMA so it never blocks the
TensorE/ScalarE pipeline of the next section.
"""

import numpy as np

D = 64
L = 4096
B = 2
V = 4
NCORES = 8
LSEC = 1024          # l columns per section (psum: [128, LSEC] f32 = 2 banks)
NSEC = L // LSEC
SCH = 128            # s-chunk (partition tile)
NSC = L // SCH

_COMPILED = None


def _build_nc():
    import concourse.bacc as bacc
    import concourse.mybir as mybir
    from concourse import tile

    f32 = mybir.dt.float32
    bf16 = mybir.dt.bfloat16
    Exp = mybir.ActivationFunctionType.Exp
    Ln = mybir.ActivationFunctionType.Ln
    add = mybir.AluOpType.add
    mult = mybir.AluOpType.mult

    nc = bacc.Bacc(
        "TRN2",
        target_bir_lowering=False,
        debug=False,
        enable_asserts=True,
        num_devices=NCORES,
    )
    x_d = nc.declare_dram_parameter("x", [D, L], f32, isOutput=False)
    xb_d = nc.declare_dram_parameter("xb", [D + 1, L], bf16, isOutput=False)
    g_d = nc.declare_dram_parameter("gaug", [D + 1, D + 1], bf16, isOutput=False)
    wv_d = nc.declare_dram_parameter("wvaug", [D + 1, D], bf16, isOutput=False)
    br_d = nc.declare_dram_parameter("bres", [D, 1], f32, isOutput=False)
    out_d = nc.declare_dram_parameter("out", [D, L], f32, isOutput=True)

    with tile.TileContext(nc) as tc:
        with (
            tc.tile_pool(name="const", bufs=1) as cpool,
            tc.tile_pool(name="big", bufs=1) as bpool,
        ):
            x_f = bpool.tile([D, L], f32)
            xbq = [
                bpool.tile(
                    [D + 1, LSEC], bf16, name=f"xbq{q}", tag=f"xbq{q}"
                )
                for q in range(NSEC)
            ]
            kp = bpool.tile([D + 1, L], bf16)
            vt = bpool.tile([128, NSC * (D + 1)], bf16)
            g_t = cpool.tile([D + 1, D + 1], bf16)
            wv_t = cpool.tile([D + 1, D], bf16)
            br_t = cpool.tile([D, 1], f32)
            warm = cpool.tile([1, 64], f32)
            warm_o = cpool.tile([1, 64], f32)
            warm_w = cpool.tile([128, 512], bf16)

            # warm the ACT table (natural_log_exp set: covers Ln AND Exp)
            # while DMAs run
            nc.vector.memset(warm[:], 1.0)
            nc.scalar.activation(warm_o[:], warm[:], Ln)
            nc.scalar.activation(warm_o[:], warm[:], Exp)

            # ---- loads ----
            nc.sync.dma_start(g_t[:], g_d[:, :])
            for q in range(NSEC):
                nc.sync.dma_start(
                    xbq[q][:], xb_d[:, q * LSEC : (q + 1) * LSEC]
                )
            nc.sync.dma_start(wv_t[:], wv_d[:, :])
            nc.sync.dma_start(br_t[:], br_d[:, :])
            nc.vector.memset(vt[:], 1.0)

            # keep the PE's HAM clock warm while DMAs land: dummy matmuls
            # on a zeroed tile (PE is otherwise idle until projections).
            nc.vector.memset(warm_w[:], 0.0)
            with tc.tile_pool(name="wps", bufs=1, space="PSUM") as wps:
                wp = wps.tile([128, 512], f32)
                for _ in range(24):
                    nc.tensor.matmul(
                        wp[:],
                        warm_w[:, 0:128],
                        warm_w[:],
                        start=True,
                        stop=True,
                    )

            # ---- projections ----
            with tc.tile_pool(name="hps", bufs=4, space="PSUM") as hps:
                # k' projection: kp[m, s] = sum_i G[i, m] x_aug[i, s]
                for c in range(8):
                    q, hh = divmod(c, 2)
                    ps = hps.tile([D + 1, 512], f32, tag="h")
                    nc.tensor.matmul(
                        ps[:],
                        g_t[:],
                        xbq[q][:, hh * 512 : (hh + 1) * 512],
                        start=True,
                        stop=True,
                    )
                    eng = nc.scalar if c % 2 == 0 else nc.vector
                    if c % 2 == 0:
                        nc.scalar.copy(kp[:, c * 512 : (c + 1) * 512], ps[:])
                    else:
                        nc.vector.tensor_copy(
                            out=kp[:, c * 512 : (c + 1) * 512], in_=ps[:]
                        )
                # vt' projection: vt[s, e] = sum_i x_aug[i, s] WV[i, e]
                for grp in range(4):
                    ps = hps.tile([128, 512], f32, tag="h")
                    for j8 in range(8):
                        j = grp * 8 + j8
                        q, r = divmod(j * SCH, LSEC)
                        nc.tensor.matmul(
                            ps[:, j8 * 64 : j8 * 64 + 64],
                            xbq[q][:, r : r + SCH],
                            wv_t[:],
                            start=True,
                            stop=True,
                        )
                    dst = (
                        vt[:, grp * 520 : (grp + 1) * 520]
                        .rearrange("p (j c) -> p j c", c=D + 1)[:, :, 0:D]
                    )
                    src = ps[:].rearrange("p (j c) -> p j c", c=D)
                    nc.vector.tensor_copy(out=dst, in_=src)

            # residual input: issue after the projections so its DMAs don't
            # delay the pipeline-critical xb/g loads (only needed by the
            # first epilogue ~50us in)
            for c in range(2):
                s = slice(c * (L // 2), (c + 1) * (L // 2))
                nc.sync.dma_start(x_f[:, s], x_d[:, s])

            # ---- attention pipeline + fused epilogue ----
            with (
                tc.tile_pool(name="stp", bufs=2, space="PSUM") as stp,
                tc.tile_pool(name="o2p", bufs=2, space="PSUM") as o2p,
                tc.tile_pool(name="atp", bufs=3) as atp,
                tc.tile_pool(name="tsb", bufs=4) as tsb,
            ):
                for sec in range(NSEC):
                    lw = sec * LSEC
                    o2 = o2p.tile([D + 1, LSEC], f32)
                    for j in range(NSC):
                        st = stp.tile([128, LSEC], f32, tag="st")
                        for h in range(LSEC // 512):
                            hs = slice(h * 512, (h + 1) * 512)
                            nc.tensor.matmul(
                                st[:, hs],
                                kp[:, j * SCH : (j + 1) * SCH],
                                xbq[sec][:, hs],
                                start=True,
                                stop=True,
                            )
                        at = atp.tile([128, LSEC], bf16)
                        nc.scalar.activation(at[:], st[:], Exp)
                        for h in range(LSEC // 512):
                            hs = slice(h * 512, (h + 1) * 512)
                            nc.tensor.matmul(
                                o2[:, hs],
                                vt[:, j * 65 : (j + 1) * 65],
                                at[:, hs],
                                start=(j == 0),
                                stop=(j == NSC - 1),
                                skip_group_check=True,
                            )
                    if sec < NSEC - 1:
                        # epilogue on DVE + GpSimd + DMA only: overlaps the
                        # next section's TensorE/ScalarE pipeline.
                        for c in range(LSEC // 512):
                            lo = lw + c * 512
                            co = slice(c * 512, (c + 1) * 512)
                            rc = tsb.tile([1, 512], f32, tag="rc")
                            nc.vector.reciprocal(rc[:], o2[D : D + 1, co])
                            rb = tsb.tile([D, 512], f32, tag="rb")
                            nc.gpsimd.partition_broadcast(rb[:], rc[:])
                            y1 = tsb.tile([D, 512], f32, tag="y1")
                            nc.vector.tensor_tensor(
                                out=y1[:], in0=o2[0:D, co], in1=rb[:], op=mult
                            )
                            res = tsb.tile([D, 512], f32, tag="res")
                            nc.vector.scalar_tensor_tensor(
                                out=res[:],
                                in0=y1[:],
                                scalar=br_t[:, 0:1],
                                in1=x_f[:, lo : lo + 512],
                                op0=add,
                                op1=add,
                            )
                            nc.sync.dma_start(out_d[:, lo : lo + 512], res[:])
                    else:
                        # last section: nothing left to overlap, so use the
                        # now-idle ScalarE for a fast reciprocal
                        # (1/d = exp(-ln(d))) instead of DVE's slow
                        # iterative-divide reciprocal.
                        tln = tsb.tile([1, LSEC], f32, tag="rc")
                        nc.scalar.activation(tln[:], o2[D : D + 1, :], Ln)
                        tlb = tsb.tile([D, LSEC], f32, tag="tlb")
                        nc.gpsimd.partition_broadcast(tlb[:], tln[:])
                        rb2 = tsb.tile([D, LSEC], f32, tag="rb2")
                        nc.scalar.activation(rb2[:], tlb[:], Exp, scale=-1.0)
                        for c in range(LSEC // 512):
                            lo = lw + c * 512
                            co = slice(c * 512, (c + 1) * 512)
                            y1 = tsb.tile([D, 512], f32, tag="y1")
                            nc.vector.tensor_tensor(
                                out=y1[:], in0=o2[0:D, co], in1=rb2[:, co], op=mult
                            )
                            res = tsb.tile([D, 512], f32, tag="res")
                            nc.vector.scalar_tensor_tensor(
                                out=res[:],
                                in0=y1[:],
                                scalar=br_t[:, 0:1],
                                in1=x_f[:, lo : lo + 512],
                                op0=add,
                                op1=add,
                            )
                            nc.sync.dma_start(out_d[:, lo : lo + 512], res[:])
    nc.compile()
    return nc


def _get_compiled():
    global _COMPILED
    if _COMPILED is None:
        _COMPILED = _build_nc()
    return _COMPILED


def _host_prep(q_v, q_g, q_b, k_v, k_g, k_b, v_v, v_g, v_b, o_v, o_g, o_b):
    import ml_dtypes

    scale = np.float32(1.0 / np.sqrt(D))

    def wn(v, g):
        v = np.asarray(v, np.float64)
        g = np.asarray(g, np.float64)
        nrm = np.sqrt((v * v).sum(1, keepdims=True))
        return (g[:, None] / nrm) * v

    wq, wk, wv, wo = wn(q_v, q_g), wn(k_v, k_g), wn(v_v, v_g), wn(o_v, o_g)
    bq = np.asarray(q_b, np.float64)
    bk = np.asarray(k_b, np.float64)
    bv = np.asarray(v_b, np.float64)
    bo = np.asarray(o_b, np.float64)

    G = np.zeros((D + 1, D + 1), np.float64)
    G[:D, :D] = (scale * wq.T @ wk).T
    G[D, :D] = scale * wq.T @ bk
    G[:D, D] = scale * wk.T @ bq
    G[D, D] = scale * (bq @ bk)

    WV = np.zeros((D + 1, D), np.float64)
    WV[:D, :] = (wo @ wv).T
    WV[D, :] = wo @ bv

    gaug = G.astype(ml_dtypes.bfloat16)
    wvaug = WV.astype(ml_dtypes.bfloat16)
    bres = np.ascontiguousarray(bo.astype(np.float32).reshape(D, 1))
    return gaug, wvaug, bres


def _make_in_maps(queries, gaug, wvaug, bres):
    import ml_dtypes

    in_maps = []
    for i in range(NCORES):
        b, h = divmod(i, V)
        x = np.ascontiguousarray(queries[b, :, :, h])  # [64, 4096] f32
        xb = np.empty((D + 1, L), ml_dtypes.bfloat16)
        xb[:D, :] = x.astype(ml_dtypes.bfloat16)
        xb[D, :] = np.ones((L,), ml_dtypes.bfloat16)
        in_maps.append(
            {"x": x, "xb": xb, "gaug": gaug, "wvaug": wvaug, "bres": bres}
        )
    return in_maps


def kernel(queries, q_v, q_g, q_b, k_v, k_g, k_b, v_v, v_g, v_b, o_v, o_g, o_b):
    from concourse.bass_utils import run_bass_kernel_spmd

    queries = np.asarray(queries, np.float32)
    gaug, wvaug, bres = _host_prep(
        q_v, q_g, q_b, k_v, k_g, k_b, v_v, v_g, v_b, o_v, o_g, o_b
    )
    in_maps = _make_in_maps(queries, gaug, wvaug, bres)

    nc = _get_compiled()
    res = run_bass_kernel_spmd(nc, in_maps, core_ids=list(range(NCORES)))

    out = np.empty((B, D, L, V), np.float32)
    for i in range(NCORES):
        b, h = divmod(i, V)
        out[b, :, :, h] = res.results[i]["out"]
    return out
